# revision 36
# baseline (speedup 1.0000x reference)
"""Trainium2 Bass kernel for CannyExtractor (NMS-suppressed canny magnitude).

Contract: kernel(x) takes FULL x [16,3,512,512] f32, returns FULL
[16,3,512,512] f32. Shards batch over 8 NeuronCores (2 images/core).

fp32 throughout (bf16/fp16/fp32r all flip too many NMS decisions vs the
reference). Design:
  - half-image elementwise ops ([128, 2, 512+] strided APs) for scheduling
    overlap; two images interleaved stage-by-stage -> 4-way parallelism
  - vertical composite convs (gauss+sobel vertical parts, gray 0.114 fold) as
    PE block-diagonal matmuls + stacked corner matmul for cross-block halos
  - horizontal composite convs as 7-tap shifted chains, with the reference's
    reflect/replicate boundary behavior folded into 3 guard columns per side
    of the gray plane (2 reflect copies + 1 solved linear-combo column)
  - NMS row-shift planes (U/D) as exact PE permutation matmuls + merged
    cross-block row DMAs
  - engine split by measured costs: DVE 2.29us/op, Pool 4.73us/op (fp32
    [128,2048]); Pool only supports add/sub/mult tensor_tensor
"""
import sys
import numpy as np

sys.path.insert(0, "/opt/trn_rl_repo")

H = W = 512
NT = 4            # 128-row blocks per image
P = 128
GP = 3            # guard cols per side on gray/u planes
WG = W + 2 * GP   # 518
WS = W + 2        # 514: s/U/D planes, 1 zero pad col per side
NI = 2            # images per core
NCORES = 8
HALVES = (slice(0, 2), slice(2, 4))

GRAY = np.array([0.299, 0.587, 0.114], np.float32)
T2 = float(np.float32((np.sqrt(2.0) - 1.0) ** 2))   # tan^2(22.5 deg)
EPS = np.float32(1e-6)


def _gauss5():
    ax = np.arange(5, dtype=np.float64) - 2.0
    g1 = np.exp(-0.5 * ax * ax)
    return g1 / g1.sum()


def _h_taps():
    """Combined 7-tap horizontal kernels (idx j+3, j=-3..3):
    d (odd: Dh o Gh, for gx), e (even: Sh o Gh, for gy)."""
    g = np.zeros(7)
    g[1:6] = _gauss5()
    d = np.zeros(7)
    e = np.zeros(7)
    for j in range(-3, 4):
        gm = g[j + 2] if 0 <= j + 2 <= 6 else 0.0
        gp_ = g[j + 4] if 0 <= j + 4 <= 6 else 0.0
        d[j + 3] = gm - gp_
        e[j + 3] = gm + 2 * g[j + 3] + gp_
    return d, e


def _guard_coeffs():
    """gp[0] = a0*g0 + a1*g1 + a2*g2 makes the 7-tap kernels reproduce the
    reference's reflect(gauss) + replicate(sobel) boundary at col 0."""
    d, _ = _h_taps()
    g = _gauss5()
    a = np.zeros(3)
    n = 8
    for k in range(3):
        u = np.zeros(n)
        u[k] = 1.0
        ur = np.concatenate([[u[2], u[1]], u, [u[n - 2], u[n - 3]]])
        blur = np.array([np.dot(g, ur[c:c + 5]) for c in range(2)])
        gx0 = blur[1] - blur[0]
        up_rest = np.array([u[2], u[1]] + list(u[0:4]))
        a[k] = (gx0 - np.dot(d[1:7], up_rest)) / d[0]
    return a


def _vert_matrix(kind):
    g1 = _gauss5()
    I = np.eye(H, dtype=np.float64)
    X = np.pad(I, ((2, 2), (0, 0)), mode="reflect")
    B = np.zeros((H, H))
    for k in range(5):
        B += g1[k] * X[k:k + H]
    Y = np.pad(B, ((1, 1), (0, 0)), mode="edge")
    taps = [1.0, 2.0, 1.0] if kind == "smooth" else [-1.0, 0.0, 1.0]
    M = np.zeros((H, H))
    for k in range(3):
        if taps[k] != 0.0:
            M += taps[k] * Y[k:k + H]
    return M


def _build_consts():
    Ms = (_vert_matrix("smooth") * float(GRAY[2])).astype(np.float32)
    Md = (_vert_matrix("diff") * float(GRAY[2])).astype(np.float32)
    vs = np.zeros((P, NT, P), np.float32)
    vd = np.zeros((P, NT, P), np.float32)
    for t in range(NT):
        vs[:, t, :] = Ms[128 * t:128 * (t + 1), 128 * t:128 * (t + 1)].T
        vd[:, t, :] = Md[128 * t:128 * (t + 1), 128 * t:128 * (t + 1)].T
    # corner weights with grayscale channel-fold: partition (c,b,k), the
    # channel ratio r_c matches gray = x0*(w0/w2) + x1*(w1/w2) + x2 (the w2
    # factor is already inside Ms/Md).
    rc = np.array([GRAY[0] / GRAY[2], GRAY[1] / GRAY[2], 1.0], np.float64)
    vcor = np.zeros((108, 2, 18), np.float32)
    for c in range(3):
        for b in range(3):
            in_rows = [128 * b + 122 + k for k in range(12)]
            out_rows = [128 * b + 125, 128 * b + 126, 128 * b + 127,
                        128 * (b + 1), 128 * (b + 1) + 1, 128 * (b + 1) + 2]
            for k, ir in enumerate(in_rows):
                for m, orr in enumerate(out_rows):
                    p = 36 * c + 12 * b + k
                    vcor[p, 0, 6 * b + m] = Ms[orr, ir] * rc[c]
                    vcor[p, 1, 6 * b + m] = Md[orr, ir] * rc[c]
    shm = np.zeros((P, 2, P), np.float32)
    for m in range(P - 1):
        shm[m + 1, 0, m] = 1.0       # U[m] = s[m+1]
    for m in range(1, P):
        shm[m - 1, 1, m] = 1.0       # D[m] = s[m-1]
    pk = np.zeros((P, 2, P), np.float32)
    pk[0, 0, 127] = 1.0    # U block t row 127 = s[t+1] row 0
    pk[127, 1, 0] = 1.0    # D block t row 0   = s[t-1] row 127
    return {"vs": vs, "vd": vd, "vcor": vcor, "shm": shm, "pk": pk}


_CACHE = {}


def _emit_image(nc, tc, pools, tens, img):
    """Generator: yields between pipeline stages so two images interleave."""
    import concourse.mybir as mybir
    AL = mybir.AluOpType
    AF = mybir.ActivationFunctionType
    F32 = mybir.dt.float32
    U8 = mybir.dt.uint8

    pwork, pmask, psmall, ppsum = pools
    xdram, ydram, c_vs, c_vd, c_vcor, c_shm, c_pk, epsb = tens

    d, e = _h_taps()
    a = _guard_coeffs()
    R01 = float(np.float32(GRAY[0] / GRAY[1]))
    R12 = float(np.float32(GRAY[1] / GRAY[2]))
    d1, d2, d3 = float(d[4]), float(d[5]), float(d[6])
    e0, e1, e2, e3 = float(e[3]), float(e[4]), float(e[5]), float(e[6])
    a0, a1c, a2c = float(a[0]), float(a[1]), float(a[2])

    INT = slice(GP, GP + W)          # interior cols of guard-padded planes
    SI = slice(1, 1 + W)             # interior cols of s/U/D planes

    def wt(w=W):
        return pwork.tile([P, NT, w], F32, tag="w", name="w")

    import os
    _upto = int(os.environ.get("KSTAGES", "99"))
    _st = [0]

    # ---- stage 0: load, gray, guard cols ----
    # corner strips prefetched straight from DRAM: raw x rows around block
    # boundaries, stacked [(c,b,k), w]; channel mix is folded into c_vcor
    xs = psmall.tile([108, W], F32, tag="xs", name="xs")
    for c in range(3):
        for b in range(3):
            nc.sync.dma_start(
                xs[36 * c + 12 * b:36 * c + 12 * b + 12, :],
                xdram[img, c, 128 * b + 122:128 * b + 134, :])
    xc = []
    for c in range(3):
        t = wt()
        nc.sync.dma_start(
            t[:], xdram[img, c].rearrange("(t p) w -> p t w", p=P))
        xc.append(t)
    gtmp = wt()
    gp = wt(WG)
    for hs in HALVES:
        nc.vector.scalar_tensor_tensor(gtmp[:, hs, :], xc[0][:, hs, :], R01,
                                       xc[1][:, hs, :], AL.mult, AL.add)
        nc.vector.scalar_tensor_tensor(gp[:, hs, INT], gtmp[:, hs, :], R12,
                                       xc[2][:, hs, :], AL.mult, AL.add)
    # reflect guards (+-1, +-2) then solved combo col (+-3), both edges at once
    nc.scalar.copy(gp[:, :, 2:3], gp[:, :, 4:5])
    nc.scalar.copy(gp[:, :, 1:2], gp[:, :, 5:6])
    nc.scalar.copy(gp[:, :, 515:516], gp[:, :, 513:514])
    nc.scalar.copy(gp[:, :, 516:517], gp[:, :, 512:513])
    tg = psmall.tile([P, NT, 2], F32, tag="tg", name="tg")
    tg2 = psmall.tile([P, NT, 2], F32, tag="tg2", name="tg2")
    nc.vector.scalar_tensor_tensor(tg[:], gp[:, :, 3:515:511], a0 / a1c,
                                   gp[:, :, 4:514:509], AL.mult, AL.add)
    nc.vector.scalar_tensor_tensor(tg2[:], tg[:], a1c / a2c,
                                   gp[:, :, 5:513:507], AL.mult, AL.add)
    nc.vector.tensor_scalar_mul(gp[:, :, 0:518:517], tg2[:], a2c)
    yield
    _st[0] += 1
    if _st[0] >= _upto:
        return

    # ---- stage 1: vertical composite convs on PE ----
    xsg = psmall.tile([108, 6], F32, tag="xsg", name="xsg")
    # reflect guards: xsg cols {1,2,3,4} <- xs cols {2,1,510,509}
    nc.scalar.copy(xsg[:, 1:2], xs[:, 2:3])
    nc.scalar.copy(xsg[:, 2:3], xs[:, 1:2])
    nc.scalar.copy(xsg[:, 3:4], xs[:, 510:511])
    nc.scalar.copy(xsg[:, 4:5], xs[:, 509:510])
    # combo guards: xsg cols {0,5} (both edges in one strided op chain)
    xg = psmall.tile([108, 2], F32, tag="xg", name="xg")
    xg2 = psmall.tile([108, 2], F32, tag="xg2", name="xg2")
    nc.vector.scalar_tensor_tensor(xg[:], xs[:, 0:512:511], a0 / a1c,
                                   xs[:, 1:511:509], AL.mult, AL.add)
    nc.vector.scalar_tensor_tensor(xg2[:], xg[:], a1c / a2c,
                                   xs[:, 2:510:507], AL.mult, AL.add)
    nc.vector.tensor_scalar_mul(xsg[:, 0:6:5], xg2[:], a2c)
    ups = []
    for ci, cm in ((0, c_vs), (1, c_vd)):
        up = wt(WG)
        for t in range(NT):
            pm = ppsum.tile([P, W], F32, tag="vm", name="vm")
            nc.tensor.matmul(pm[:], cm[:, t, :], gp[:, t, INT],
                             start=True, stop=True)
            nc.scalar.copy(up[:, t, INT], pm[:])
        pg = ppsum.tile([P, 24], F32, tag="sm", name="sm")
        for t in range(NT):
            nc.tensor.matmul(pg[:, 6 * t:6 * t + 3], cm[:, t, :],
                             gp[:, t, 0:3], start=True, stop=True)
            nc.tensor.matmul(pg[:, 6 * t + 3:6 * t + 6], cm[:, t, :],
                             gp[:, t, 515:518], start=True, stop=True)
        pgv = pg[:].rearrange("p (t x) -> p t x", t=NT)
        nc.scalar.copy(up[:, :, 0:3], pgv[:, :, 0:3])
        nc.scalar.copy(up[:, :, 515:518], pgv[:, :, 3:6])
        # corner fix: 3 rows each side of the block boundaries
        cps = ppsum.tile([18, W], F32, tag="sm", name="sm")
        nc.tensor.matmul(cps[:], c_vcor[:, ci, :], xs[:], start=True,
                         stop=True)
        cpg = ppsum.tile([18, 6], F32, tag="sm", name="sm")
        nc.tensor.matmul(cpg[:, 0:3], c_vcor[:, ci, :], xsg[:, 0:3],
                         start=True, stop=True)
        nc.tensor.matmul(cpg[:, 3:6], c_vcor[:, ci, :], xsg[:, 3:6],
                         start=True, stop=True)
        co = psmall.tile([18, WG], F32, tag="co", name="co")
        nc.scalar.copy(co[:, INT], cps[:])
        nc.scalar.copy(co[:, 0:3], cpg[:, 0:3])
        nc.scalar.copy(co[:, 515:518], cpg[:, 3:6])
        for b in range(3):
            nc.sync.dma_start(up[125:128, b, :], co[6 * b:6 * b + 3, :])
            nc.sync.dma_start(up[0:3, b + 1, :], co[6 * b + 3:6 * b + 6, :])
        ups.append(up)
    u1p, u2p = ups
    yield
    _st[0] += 1
    if _st[0] >= _upto:
        return

    # ---- stage 2: horizontal 7-tap chains (per half) ----
    s1 = wt()
    s2 = wt()
    s3 = wt()
    q1 = wt()
    gxp = wt()
    b1 = wt()
    b2 = wt()
    b3 = wt()
    qy = wt()
    ry = wt()
    gyp = wt()
    for hs in HALVES:
        nc.gpsimd.tensor_tensor(s1[:, hs, :], u1p[:, hs, 4:516],
                                u1p[:, hs, 2:514], AL.subtract)
        nc.gpsimd.tensor_tensor(s2[:, hs, :], u1p[:, hs, 5:517],
                                u1p[:, hs, 1:513], AL.subtract)
        nc.gpsimd.tensor_tensor(s3[:, hs, :], u1p[:, hs, 6:518],
                                u1p[:, hs, 0:512], AL.subtract)
        nc.vector.scalar_tensor_tensor(q1[:, hs, :], s3[:, hs, :], d3 / d2,
                                       s2[:, hs, :], AL.mult, AL.add)
        nc.vector.scalar_tensor_tensor(gxp[:, hs, :], q1[:, hs, :], d2 / d1,
                                       s1[:, hs, :], AL.mult, AL.add)
        yield
        _st[0] += 1
        if _st[0] >= _upto:
            return
        nc.gpsimd.tensor_tensor(b1[:, hs, :], u2p[:, hs, 4:516],
                                u2p[:, hs, 2:514], AL.add)
        nc.gpsimd.tensor_tensor(b2[:, hs, :], u2p[:, hs, 5:517],
                                u2p[:, hs, 1:513], AL.add)
        nc.gpsimd.tensor_tensor(b3[:, hs, :], u2p[:, hs, 6:518],
                                u2p[:, hs, 0:512], AL.add)
        nc.vector.scalar_tensor_tensor(qy[:, hs, :], b3[:, hs, :], e3 / e2,
                                       b2[:, hs, :], AL.mult, AL.add)
        nc.vector.scalar_tensor_tensor(ry[:, hs, :], qy[:, hs, :], e2 / e1,
                                       b1[:, hs, :], AL.mult, AL.add)
        nc.vector.scalar_tensor_tensor(gyp[:, hs, :], ry[:, hs, :], e1 / e0,
                                       u2p[:, hs, INT], AL.mult, AL.add)
        yield
        _st[0] += 1
        if _st[0] >= _upto:
            return

    # ---- stage 3: squares, s-plane, direction masks (per half) ----
    sqx = wt()
    sqy = wt()
    s = wt(WS)
    pxy = wt()
    md1 = pmask.tile([P, NT, W], U8, tag="m", name="m")
    ch = pmask.tile([P, NT, W], U8, tag="m", name="m")
    cv = pmask.tile([P, NT, W], U8, tag="m", name="m")
    nc.gpsimd.memset(s[:, :, 0:514:513], 0.0)
    for hs in HALVES:
        nc.scalar.activation(sqx[:, hs, :], gxp[:, hs, :], AF.Square, 0.0, d1)
        nc.scalar.activation(sqy[:, hs, :], gyp[:, hs, :], AF.Square, 0.0, e0)
        nc.gpsimd.tensor_tensor(s[:, hs, SI], sqx[:, hs, :], sqy[:, hs, :],
                                AL.add)
        nc.gpsimd.tensor_tensor(pxy[:, hs, :], gxp[:, hs, :], gyp[:, hs, :],
                                AL.mult)
        nc.vector.tensor_scalar(md1[:, hs, :], pxy[:, hs, :], 0.0, None,
                                AL.is_gt)
        nc.vector.scalar_tensor_tensor(ch[:, hs, :], sqx[:, hs, :], T2,
                                       sqy[:, hs, :], AL.mult, AL.is_ge)
        nc.vector.scalar_tensor_tensor(cv[:, hs, :], sqy[:, hs, :], T2,
                                       sqx[:, hs, :], AL.mult, AL.is_gt)
        yield
        _st[0] += 1
        if _st[0] >= _upto:
            return
    _st[0] += 1
    if _st[0] >= _upto:
        return

    # ---- stage 4: U/D row-shift planes via PE permutation ----
    U = wt(WS)
    D = wt(WS)
    for pl, ci in ((U, 0), (D, 1)):
        nc.gpsimd.memset(pl[:, :, 0:514:513], 0.0)
        for t in range(NT):
            tn = t + 1 if ci == 0 else t - 1
            has_nb = 0 <= tn < NT
            pm = ppsum.tile([P, W], F32, tag="vm", name="vm")
            nc.tensor.matmul(pm[:], c_shm[:, ci, :], s[:, t, SI],
                             start=True, stop=not has_nb)
            if has_nb:
                nc.tensor.matmul(pm[:], c_pk[:, ci, :], s[:, tn, SI],
                                 start=False, stop=True)
            nc.scalar.copy(pl[:, t, SI], pm[:])
    yield
    _st[0] += 1
    if _st[0] >= _upto:
        return

    # ---- stage 5: NMS select, suppress, output (per half) ----
    mh = wt()
    mv = wt()
    md1m = wt()
    sel = wt()
    keep = wt()
    mag = wt()
    magc = wt()
    out_ = wt()
    for hs in HALVES:
        nc.vector.tensor_tensor(mh[:, hs, :], s[:, hs, 0:512],
                                s[:, hs, 2:514], AL.max)
        nc.vector.tensor_tensor(mv[:, hs, :], U[:, hs, SI], D[:, hs, SI],
                                AL.max)
        nc.vector.tensor_tensor(md1m[:, hs, :], U[:, hs, 2:514],
                                D[:, hs, 0:512], AL.max)
        nc.vector.tensor_tensor(sel[:, hs, :], U[:, hs, 0:512],
                                D[:, hs, 2:514], AL.max)
        nc.vector.copy_predicated(sel[:, hs, :], md1[:, hs, :],
                                  md1m[:, hs, :])
        nc.vector.copy_predicated(sel[:, hs, :], cv[:, hs, :], mv[:, hs, :])
        nc.vector.copy_predicated(sel[:, hs, :], ch[:, hs, :], mh[:, hs, :])
        nc.vector.tensor_tensor(keep[:, hs, :], s[:, hs, SI], sel[:, hs, :],
                                AL.is_gt)
        nc.scalar.activation(mag[:, hs, :], s[:, hs, SI], AF.Sqrt,
                             epsb[:], 1.0)
        nc.vector.tensor_scalar_min(magc[:, hs, :], mag[:, hs, :], 1.0)
        nc.gpsimd.tensor_tensor(out_[:, hs, :], magc[:, hs, :],
                                keep[:, hs, :], AL.mult)
        for t in range(hs.start, hs.stop):
            nc.gpsimd.dma_start(
                ydram[img].rearrange("c (t p) w -> p c t w", p=P)[:, :, t, :],
                out_[:, t, :].unsqueeze(1).broadcast_to([P, 3, W]))
        yield
        _st[0] += 1
        if _st[0] >= _upto:
            return
    _st[0] += 1
    if _st[0] >= _upto:
        return


def _build(nloop=1):
    import concourse.bacc as bacc
    import concourse.mybir as mybir
    from concourse import tile
    from contextlib import nullcontext
    F32 = mybir.dt.float32

    nc = bacc.Bacc("TRN2", target_bir_lowering=False, debug=False,
                   num_devices=NCORES)
    xdram = nc.declare_dram_parameter("xc", [NI, 3, H, W], F32, isOutput=False)
    c_vs_d = nc.declare_dram_parameter("vs", [P, NT, P], F32, isOutput=False)
    c_vd_d = nc.declare_dram_parameter("vd", [P, NT, P], F32, isOutput=False)
    c_vcor_d = nc.declare_dram_parameter("vcor", [108, 2, 18], F32,
                                         isOutput=False)
    c_shm_d = nc.declare_dram_parameter("shm", [P, 2, P], F32, isOutput=False)
    c_pk_d = nc.declare_dram_parameter("pk", [P, 2, P], F32, isOutput=False)
    ydram = nc.declare_dram_parameter("y", [NI, 3, H, W], F32, isOutput=True)

    with tile.TileContext(nc) as tc:
        with tc.tile_pool(name="pconst", bufs=1) as pconst, \
             tc.tile_pool(name="pwork", bufs=19) as pwork, \
             tc.tile_pool(name="pmask", bufs=6) as pmask, \
             tc.tile_pool(name="psmall", bufs=2) as psmall, \
             tc.tile_pool(name="ppsum", bufs=4, space="PSUM") as ppsum:
            c_vs = pconst.tile([P, NT, P], F32, tag="cvs")
            nc.sync.dma_start(c_vs[:], c_vs_d[:])
            c_vd = pconst.tile([P, NT, P], F32, tag="cvd")
            nc.sync.dma_start(c_vd[:], c_vd_d[:])
            c_vcor = pconst.tile([108, 2, 18], F32, tag="cvcor")
            nc.sync.dma_start(c_vcor[:], c_vcor_d[:])
            c_shm = pconst.tile([P, 2, P], F32, tag="cshm")
            nc.sync.dma_start(c_shm[:], c_shm_d[:])
            c_pk = pconst.tile([P, 2, P], F32, tag="cpk")
            nc.sync.dma_start(c_pk[:], c_pk_d[:])
            epsb = pconst.tile([P, 1], F32, tag="epsb")
            nc.gpsimd.memset(epsb[:], float(EPS))

            pools = (pwork, pmask, psmall, ppsum)
            tens = (xdram, ydram, c_vs, c_vd, c_vcor, c_shm, c_pk, epsb)
            loop_cm = tc.For_i(0, nloop, 1) if nloop > 1 else nullcontext()
            with loop_cm:
                gens = [_emit_image(nc, tc, pools, tens, img)
                        for img in range(NI)]
                done = [False] * NI
                while not all(done):
                    for i, gi in enumerate(gens):
                        if not done[i]:
                            try:
                                next(gi)
                            except StopIteration:
                                done[i] = True

    nc.compile()
    return nc


def _get_nc():
    if "nc" not in _CACHE:
        _CACHE["nc"] = _build()
        _CACHE["consts"] = _build_consts()
    return _CACHE["nc"], _CACHE["consts"]


def kernel(x):
    from concourse.bass_utils import run_bass_kernel_spmd
    x = np.ascontiguousarray(np.asarray(x), dtype=np.float32)
    assert x.shape == (16, 3, H, W), x.shape
    nc, consts = _get_nc()
    in_maps = []
    for c in range(NCORES):
        m = {"xc": x[NI * c:NI * (c + 1)]}
        m.update(consts)
        in_maps.append(m)
    res = run_bass_kernel_spmd(nc, in_maps, list(range(NCORES)))
    y = np.concatenate([res.results[c]["y"] for c in range(NCORES)], axis=0)
    return y.astype(np.float32)


if __name__ == "__main__":
    import golden
    x = np.load("/root/problem/x_cache.npy")
    y = kernel(x)
    ref = golden.reference_np(x)
    d = y - ref
    print("L2 rel:", np.linalg.norm(d) / np.linalg.norm(ref))
    print("absmax:", np.abs(d).max(), " bigpix:", (np.abs(d) > 1e-3).sum())
